# revision 42
# baseline (speedup 1.0000x reference)
"""MoE layer (top-2 of 8 experts, H=1024, FFN=4096) on 8 TRN2 NeuronCores.

Expert-parallel: core e holds expert e's weights resident in SBUF. The
(tiny) router runs on host; tokens are gathered per-expert into
capacity-padded batches, each core runs the expert FFN and the host
applies the gate weight and scatter-adds the two expert contributions.

Device layout per core (C = per-expert token capacity, multiple of 8),
tokens processed in PSUM-sized chunks of <=256:
  GEMM1  h[f, c] = w1s[h, f].T @ x[h, c]    (F on partitions, tokens free)
  GEMM2  y[h, c] = w2s[f, h].T @ h[f, c]    (H on partitions, tokens free)
Both GEMMs stream the token axis, so compute scales with C exactly (no
128-ceil on token tiles). y accumulates in PSUM across all 32 F-tiles
(8 slabs x 4): 8 accumulators packed 2-per-bank; since start=True on a
matmul clears the whole PSUM bank, only the first matmul per BANK uses
start=True — the bank-mate's first matmul relies on the per-element
has_written bits to overwrite. Eviction is one ScalarE Identity per
H-tile folding b2 in as a per-partition bias; gate + top-2 combine on
host.

Weights/x are staged in DRAM pre-swizzled to match SBUF layout exactly
(slab-major, partition rows contiguous) so weight DMA moves in 2KB+
packets; v2 measured 256B packets on w1 and starved the PE for ~15us.

GEMMs run in bf16 (fp32 matmul is 4x slower; fp8 DoubleRow fails the
2e-2 accuracy gate: each quantized tensor alone contributes ~2.7e-2).
"""

import os

os.environ.setdefault("NEURON_RT_RESET_CORES", "1")

import ml_dtypes
import numpy as np

import concourse.bass as bass  # noqa: F401  (bass types via bacc)
import concourse.mybir as mybir
from concourse import bacc
from concourse.tile import TileContext
from concourse.bass_utils import run_bass_kernel_spmd

H = 1024
E = 8
F = 4096
TOPK = 2
P = 128
N_CORES = 8
FP32 = mybir.dt.float32
BF16 = mybir.dt.bfloat16

NTH = 8            # F slabs
FT = F // NTH      # 512 F columns per slab
MF = FT // P       # 4 f-tiles of 128 per slab
KH = H // P        # 8 contraction tiles for GEMM1
HT = H // P        # 8 output H-tiles for GEMM2

_cache: dict = {}

# Test-harness knobs (harness-safe defaults): set TRACE=True before calling
# kernel() to profile the device run; exec time lands in LAST_EXEC_TIME_NS.
TRACE = False
LAST_EXEC_TIME_NS = None


def _chunks(C: int):
    """Token chunks, multiples of 8, each <=512. The final chunk is kept
    at ~104 tokens so the last eviction+store drain is short; 104 stays
    above the PE's ~60-cycle issue floor and the ScalarE fixed cost per
    gelu, which made a 64-token tail a net loss."""
    if C <= 512:
        return [(0, C)]
    tail = 104
    rest = C - tail
    nch = -(-rest // 512)
    u = rest // 8
    units = [u // nch + (1 if i < u % nch else 0) for i in range(nch)]
    widths = [un * 8 for un in units] + [tail]
    assert sum(widths) == C and all(0 < w <= 512 for w in widths)
    out = []
    off = 0
    for w in widths:
        out.append((off, w))
        off += w
    return out


def _build(C: int):
    """Build + compile the per-core expert-FFN program for capacity C."""
    assert C % 8 == 0
    cbs = _chunks(C)

    nc = bacc.Bacc("TRN2", target_bir_lowering=False, debug=False,
                   num_devices=N_CORES)

    # All big inputs pre-swizzled on host to the exact SBUF layout:
    # w1s row th*128+p, col m*1024 + k*128 + f2 = w1.T[k*128+p, th*512+m*128+f2]
    # w2s row th*128+p, col m*1024+h  = w2.T[(th*4+m)*128+p, h]
    # xc  row p,        col 8*coff + k*ck + c = x[coff+c, k*128+p]
    # per-slab concatenation of w1 and w2 so one DMA trigger (~600ns of SP
    # issue time each, flat regardless of size) loads a whole slab
    WS1 = KH * FT                  # 4096 w1 cols per slab
    WSL = WS1 + MF * H             # + 4096 w2 cols
    wall = nc.dram_tensor("wall", [NTH * P, WSL], BF16, kind="ExternalInput")
    xc = nc.dram_tensor("xc", [P, KH * C], BF16, kind="ExternalInput")
    b1c = nc.dram_tensor("b1c", [P, F // P], FP32, kind="ExternalInput")
    b2c = nc.dram_tensor("b2c", [P, H // P], FP32, kind="ExternalInput")
    out = nc.dram_tensor("out", [H, C], FP32, kind="ExternalOutput")

    out_v = out.rearrange("(t p) c -> p t c", p=P)   # [128, 8, C]

    GELU = mybir.ActivationFunctionType.Gelu

    with TileContext(nc) as tc:
        with (
            tc.tile_pool(name="const", bufs=1) as constp,
            tc.tile_pool(name="xp", bufs=1) as xp,
            tc.tile_pool(name="wp", bufs=1) as wp,
            tc.tile_pool(name="hp", bufs=1) as hp,
            tc.tile_pool(name="op", bufs=2) as op,
            tc.tile_pool(name="ps1", bufs=2, space="PSUM") as ps1p,
            tc.tile_pool(name="psy", bufs=1, space="PSUM") as psyp,
        ):
            # PE warmup: the NEFF prologue keeps DMA dead for ~10us; a run
            # of dummy matmuls on a memset tile keeps the PE HAM-warm from
            # t~0 so the first real matmuls issue at full clock.
            zt = constp.tile([P, 2 * P], BF16, tag="zt")
            nc.vector.memset(zt[:], 0.0)
            wups = psyp.tile([P, 2, 256], FP32, tag="warm")
            for i in range(72):
                # rotate targets so the Tile WAW chain is 2 matmuls deep
                nc.tensor.matmul(wups[:, i % 2, :], zt[:, :P], zt[:],
                                 start=True, stop=True)

            # DMA emission order = arrival order: consts, x chunk 0 split
            # per k-tile + slab-0 w1 split per f-tile (first GEMM1 group
            # only needs ~0.3MB), then slab 0..7 weights, then the
            # remaining x chunks (not needed until ~50us in).
            b1_sb = constp.tile([P, F // P], FP32, tag="b1")
            nc.sync.dma_start(out=b1_sb[:], in_=b1c[:])
            b2_sb = constp.tile([P, H // P], FP32, tag="b2")
            nc.sync.dma_start(out=b2_sb[:], in_=b2c[:])

            x_t = []

            def load_x(ci):
                coff, ck = cbs[ci]
                t = xp.tile([P, KH, ck], BF16, tag=f"x{ci}", name=f"x{ci}")
                nc.sync.dma_start(
                    out=t[:], in_=xc[:, KH * coff:KH * (coff + ck)])
                x_t.append(t)

            load_x(0)

            # DMA arrival order matches consumption order: all w1 slabs
            # (GEMM1 phase of chunk 0) before all w2 slabs (GEMM2 phase).
            # Slab 0's w1 lands in f-tile quarters so the first GEMM1
            # group starts after ~0.3MB.
            w_sb = []
            for th in range(NTH):
                t = wp.tile([P, WSL], BF16, tag=f"w_{th}", name=f"w_{th}")
                rows = wall[th * P:(th + 1) * P, :]
                if th == 0:
                    for m in range(MF):
                        nc.sync.dma_start(
                            out=t[:, m * KH * P:(m + 1) * KH * P],
                            in_=rows[:, m * KH * P:(m + 1) * KH * P])
                else:
                    nc.sync.dma_start(out=t[:, :WS1], in_=rows[:, :WS1])
                w_sb.append(t)
            for th in range(NTH):
                rows = wall[th * P:(th + 1) * P, :]
                nc.sync.dma_start(out=w_sb[th][:, WS1:], in_=rows[:, WS1:])

            def w1sl(th, m, k):
                return w_sb[th][:, m * KH * P + k * P:m * KH * P + (k + 1) * P]

            def w2sl(th, m, ht):
                base = WS1 + m * H + ht * P
                return w_sb[th][:, base:base + P]

            for ci in range(1, len(cbs)):
                load_x(ci)

            for ci, (coff, ck) in enumerate(cbs):
                last_chunk = ci == len(cbs) - 1

                # 4 full-bank y accumulators, reused by the two GEMM2
                # half-H passes
                def ytile(q, half):
                    return psyp.tile([P, 512], FP32, tag=f"y{q}",
                                     name=f"y{q}_{ci}_{half}")

                # evictions go to DVE (ScalarE is the gelu critical path)
                # into a staging tile; one store DMA per half-H pass
                def o4tile(half):
                    return op.tile([P, 4, 512], FP32, tag="o4",
                                   name=f"o4_{ci}_{half}")

                # GEMM1 phase: h for all 8 slabs staged in SBUF
                hL = hp.tile([P, NTH, MF, 512], BF16, tag="h",
                             name=f"h_{ci}")
                for th in range(NTH):
                    for m in range(MF):
                        pt = ps1p.tile([P, 512], FP32, tag="ps1")
                        for k in range(KH):
                            nc.tensor.matmul(
                                pt[:, :ck],
                                w1sl(th, m, k),
                                x_t[ci][:, k, :],
                                start=(k == 0), stop=(k == KH - 1),
                            )
                        nc.scalar.activation(
                            hL[:, th, m, :ck], pt[:, :ck], GELU,
                            bias=b1_sb[:, th * MF + m:th * MF + m + 1],
                        )

                # GEMM2: two passes of 4 H-tiles. th outer inside a pass
                # so the w2 slabs stream at the DMA-sustainable rate on
                # chunk 0; on the final chunk's second pass, ht outer so
                # each accumulator closes early and evictions + stores
                # chase the remaining matmuls
                for half in range(2):
                    y_q = [ytile(q, half) for q in range(4)]
                    o4 = o4tile(half)

                    def evict(q):
                        nc.vector.tensor_scalar_add(
                            o4[:, q, :ck], y_q[q][:, :ck],
                            b2_sb[:, 4 * half + q:4 * half + q + 1])

                    if last_chunk and half == 1:
                        # per-ht stores: a single merged store at the very
                        # end exposes its whole ~700KB transfer (~4us)
                        for q in range(4):
                            ht = 4 * half + q
                            for th in range(NTH):
                                for m in range(MF):
                                    nc.tensor.matmul(
                                        y_q[q][:, :ck],
                                        w2sl(th, m, ht),
                                        hL[:, th, m, :ck],
                                        start=(th == 0 and m == 0),
                                        stop=(th == NTH - 1 and m == MF - 1),
                                    )
                            evict(q)
                            nc.sync.dma_start(
                                out=out_v[:, ht:ht + 1, coff:coff + ck],
                                in_=o4[:, q:q + 1, :ck])
                        continue
                    else:
                        for th in range(NTH):
                            for m in range(MF):
                                for q in range(4):
                                    nc.tensor.matmul(
                                        y_q[q][:, :ck],
                                        w2sl(th, m, 4 * half + q),
                                        hL[:, th, m, :ck],
                                        start=(th == 0 and m == 0),
                                        stop=(th == NTH - 1 and m == MF - 1),
                                    )
                        for q in range(4):
                            evict(q)
                    nc.sync.dma_start(
                        out=out_v[:, 4 * half:4 * half + 4, coff:coff + ck],
                        in_=o4[:, :, :ck])

    nc.compile()
    return nc


def _route(x: np.ndarray, router_w: np.ndarray):
    """Host router: top-2 expert ids + softmax gates per token."""
    logits = x @ router_w.T                                   # [T, E]
    top_i = np.argsort(-logits, axis=1, kind="stable")[:, :TOPK]
    top_v = np.take_along_axis(logits, top_i, axis=1)
    mx = top_v.max(axis=1, keepdims=True)
    ex = np.exp(top_v - mx)
    rw = ex / ex.sum(axis=1, keepdims=True)
    return top_i, rw.astype(np.float32)


def _swizzle_wall(w1e: np.ndarray, w2e: np.ndarray) -> np.ndarray:
    # w1 [F, H] -> rows th*128+p, cols m*1024 + k*128 + f2
    a = w1e.reshape(NTH, MF, P, KH, P).transpose(0, 4, 1, 3, 2).reshape(
        NTH * P, KH * FT)
    # w2 [H, F] -> w2.T [F, H] -> rows th*128+p, cols m*1024+h
    b = w2e.T.reshape(NTH, MF, P, H).transpose(0, 2, 1, 3).reshape(
        NTH * P, MF * H)
    return np.ascontiguousarray(
        np.concatenate([a, b], axis=1)).astype(ml_dtypes.bfloat16)


def kernel(hidden_states, router_w, w1, b1, w2, b2):
    hidden_states = np.ascontiguousarray(np.asarray(hidden_states, np.float32))
    router_w = np.ascontiguousarray(np.asarray(router_w, np.float32))
    w1 = np.asarray(w1, np.float32)
    b1 = np.asarray(b1, np.float32)
    w2 = np.asarray(w2, np.float32)
    b2 = np.asarray(b2, np.float32)

    B, S, _ = hidden_states.shape
    T = B * S
    x = hidden_states.reshape(T, H)

    top_i, rw = _route(x, router_w)

    sel_idx = []
    sel_gate = []
    for e in range(E):
        mask = top_i == e                                     # [T, K]
        rows = np.nonzero(mask.any(axis=1))[0]
        g = rw[rows[:, None], np.argmax(mask[rows], axis=1)[:, None]][:, 0]
        sel_idx.append(rows)
        sel_gate.append(g.astype(np.float32))

    # One job per (expert, token-chunk). Normally each expert fits in one
    # chunk and a single 8-core SPMD round runs everything; with an extreme
    # routing skew an expert's batch is split into <=C_MAX chunks (bounded
    # by SBUF) and extra rounds run.
    C_MAX = 2048
    jobs = []                                   # (expert, rows, gates)
    for e in range(E):
        rows, g = sel_idx[e], sel_gate[e]
        for off in range(0, max(len(rows), 1), C_MAX):
            jobs.append((e, rows[off:off + C_MAX], g[off:off + C_MAX]))

    n_rounds = -(-len(jobs) // N_CORES)
    cmax = max(len(r) for _, r, _ in jobs)
    C = max(P, -(-cmax // 8) * 8)

    if C not in _cache:
        _cache[C] = _build(C)
    nc = _cache[C]
    cbs = _chunks(C)

    w_bf = {}
    def expert_inputs(e):
        if e not in w_bf:
            w_bf[e] = {
                "wall": _swizzle_wall(w1[e], w2[e]),
                "b1c": np.ascontiguousarray(b1[e].reshape(F // P, P).T),
                "b2c": np.ascontiguousarray(b2[e].reshape(H // P, P).T),
            }
        return w_bf[e]

    global LAST_EXEC_TIME_NS
    LAST_EXEC_TIME_NS = 0
    out = np.zeros((T, H), np.float32)
    for r in range(n_rounds):
        batch = jobs[r * N_CORES:(r + 1) * N_CORES]
        while len(batch) < N_CORES:
            batch.append((0, sel_idx[0][:0], sel_gate[0][:0]))
        in_maps = []
        for e, rows, g in batch:
            n_e = len(rows)
            xpad = np.zeros((C, H), np.float32)
            xpad[:n_e] = x[rows]
            # chunk-major swizzle: col 8*coff + k*ck + c = x[coff+c, k*128+p]
            xc = np.concatenate(
                [xpad[coff:coff + ck].reshape(ck, KH, P)
                 .transpose(2, 1, 0).reshape(P, KH * ck)
                 for coff, ck in cbs], axis=1)
            in_maps.append({
                "xc": np.ascontiguousarray(xc).astype(ml_dtypes.bfloat16),
                **expert_inputs(e),
            })

        res = run_bass_kernel_spmd(nc, in_maps, list(range(N_CORES)), trace=TRACE)
        if res.exec_time_ns:
            LAST_EXEC_TIME_NS += res.exec_time_ns

        for core, (e, rows, g) in enumerate(batch):
            if len(rows):
                # y comes back [H, C] with b2 folded in; gate + top-2
                # combine on host (row indices unique within one job)
                out[rows] += g[:, None] * res.results[core]["out"][:, :len(rows)].T
    return out.reshape(B, S, H)


# revision 43
# speedup vs baseline: 1.0050x; 1.0050x over previous
"""MoE layer (top-2 of 8 experts, H=1024, FFN=4096) on 8 TRN2 NeuronCores.

Expert-parallel: core e holds expert e's weights resident in SBUF. The
(tiny) router runs on host; tokens are gathered per-expert into
capacity-padded batches, each core runs the expert FFN and the host
applies the gate weight and scatter-adds the two expert contributions.

Device layout per core (C = per-expert token capacity, multiple of 8),
tokens processed in PSUM-sized chunks of <=256:
  GEMM1  h[f, c] = w1s[h, f].T @ x[h, c]    (F on partitions, tokens free)
  GEMM2  y[h, c] = w2s[f, h].T @ h[f, c]    (H on partitions, tokens free)
Both GEMMs stream the token axis, so compute scales with C exactly (no
128-ceil on token tiles). y accumulates in PSUM across all 32 F-tiles
(8 slabs x 4): 8 accumulators packed 2-per-bank; since start=True on a
matmul clears the whole PSUM bank, only the first matmul per BANK uses
start=True — the bank-mate's first matmul relies on the per-element
has_written bits to overwrite. Eviction is one ScalarE Identity per
H-tile folding b2 in as a per-partition bias; gate + top-2 combine on
host.

Weights/x are staged in DRAM pre-swizzled to match SBUF layout exactly
(slab-major, partition rows contiguous) so weight DMA moves in 2KB+
packets; v2 measured 256B packets on w1 and starved the PE for ~15us.

GEMMs run in bf16 (fp32 matmul is 4x slower; fp8 DoubleRow fails the
2e-2 accuracy gate: each quantized tensor alone contributes ~2.7e-2).
"""

import os

os.environ.setdefault("NEURON_RT_RESET_CORES", "1")

import ml_dtypes
import numpy as np

import concourse.bass as bass  # noqa: F401  (bass types via bacc)
import concourse.mybir as mybir
from concourse import bacc
from concourse.tile import TileContext
from concourse.bass_utils import run_bass_kernel_spmd

H = 1024
E = 8
F = 4096
TOPK = 2
P = 128
N_CORES = 8
FP32 = mybir.dt.float32
BF16 = mybir.dt.bfloat16

NTH = 8            # F slabs
FT = F // NTH      # 512 F columns per slab
MF = FT // P       # 4 f-tiles of 128 per slab
KH = H // P        # 8 contraction tiles for GEMM1
HT = H // P        # 8 output H-tiles for GEMM2

_cache: dict = {}

# Test-harness knobs (harness-safe defaults): set TRACE=True before calling
# kernel() to profile the device run; exec time lands in LAST_EXEC_TIME_NS.
TRACE = False
LAST_EXEC_TIME_NS = None


def _chunks(C: int):
    """Near-even token chunks, multiples of 8, each <=512. (A small
    final chunk to shorten the drain was tried and lost more to per-chunk
    gelu/issue overheads than it saved.)"""
    nch = -(-C // 512)
    u = C // 8
    units = [u // nch + (1 if i < u % nch else 0) for i in range(nch)]
    widths = [un * 8 for un in units]
    assert sum(widths) == C and all(0 < w <= 512 for w in widths)
    out = []
    off = 0
    for w in widths:
        out.append((off, w))
        off += w
    return out


def _build(C: int):
    """Build + compile the per-core expert-FFN program for capacity C."""
    assert C % 8 == 0
    cbs = _chunks(C)

    nc = bacc.Bacc("TRN2", target_bir_lowering=False, debug=False,
                   num_devices=N_CORES)

    # All big inputs pre-swizzled on host to the exact SBUF layout:
    # w1s row th*128+p, col m*1024 + k*128 + f2 = w1.T[k*128+p, th*512+m*128+f2]
    # w2s row th*128+p, col m*1024+h  = w2.T[(th*4+m)*128+p, h]
    # xc  row p,        col 8*coff + k*ck + c = x[coff+c, k*128+p]
    # per-slab concatenation of w1 and w2 so one DMA trigger (~600ns of SP
    # issue time each, flat regardless of size) loads a whole slab
    WS1 = KH * FT                  # 4096 w1 cols per slab
    WSL = WS1 + MF * H             # + 4096 w2 cols
    wall = nc.dram_tensor("wall", [NTH * P, WSL], BF16, kind="ExternalInput")
    xc = nc.dram_tensor("xc", [P, KH * C], BF16, kind="ExternalInput")
    b1c = nc.dram_tensor("b1c", [P, F // P], FP32, kind="ExternalInput")
    b2c = nc.dram_tensor("b2c", [P, H // P], FP32, kind="ExternalInput")
    out = nc.dram_tensor("out", [H, C], FP32, kind="ExternalOutput")

    out_v = out.rearrange("(t p) c -> p t c", p=P)   # [128, 8, C]

    GELU = mybir.ActivationFunctionType.Gelu

    with TileContext(nc) as tc:
        with (
            tc.tile_pool(name="const", bufs=1) as constp,
            tc.tile_pool(name="xp", bufs=1) as xp,
            tc.tile_pool(name="wp", bufs=1) as wp,
            tc.tile_pool(name="hp", bufs=1) as hp,
            tc.tile_pool(name="op", bufs=2) as op,
            tc.tile_pool(name="ps1", bufs=2, space="PSUM") as ps1p,
            tc.tile_pool(name="psy", bufs=1, space="PSUM") as psyp,
        ):
            # PE warmup: the NEFF prologue keeps DMA dead for ~10us; a run
            # of dummy matmuls on a memset tile keeps the PE HAM-warm from
            # t~0 so the first real matmuls issue at full clock.
            zt = constp.tile([P, 2 * P], BF16, tag="zt")
            nc.vector.memset(zt[:], 0.0)
            wups = psyp.tile([P, 2, 256], FP32, tag="warm")
            for i in range(72):
                # rotate targets so the Tile WAW chain is 2 matmuls deep
                nc.tensor.matmul(wups[:, i % 2, :], zt[:, :P], zt[:],
                                 start=True, stop=True)

            # DMA emission order = arrival order: consts, x chunk 0 split
            # per k-tile + slab-0 w1 split per f-tile (first GEMM1 group
            # only needs ~0.3MB), then slab 0..7 weights, then the
            # remaining x chunks (not needed until ~50us in).
            b1_sb = constp.tile([P, F // P], FP32, tag="b1")
            nc.sync.dma_start(out=b1_sb[:], in_=b1c[:])
            b2_sb = constp.tile([P, H // P], FP32, tag="b2")
            nc.sync.dma_start(out=b2_sb[:], in_=b2c[:])

            x_t = []

            def load_x(ci):
                coff, ck = cbs[ci]
                t = xp.tile([P, KH, ck], BF16, tag=f"x{ci}", name=f"x{ci}")
                nc.sync.dma_start(
                    out=t[:], in_=xc[:, KH * coff:KH * (coff + ck)])
                x_t.append(t)

            load_x(0)

            # DMA arrival order matches consumption order: all w1 slabs
            # (GEMM1 phase of chunk 0) before all w2 slabs (GEMM2 phase).
            # Slab 0's w1 lands in f-tile quarters so the first GEMM1
            # group starts after ~0.3MB.
            w_sb = []
            for th in range(NTH):
                t = wp.tile([P, WSL], BF16, tag=f"w_{th}", name=f"w_{th}")
                rows = wall[th * P:(th + 1) * P, :]
                if th == 0:
                    for m in range(MF):
                        nc.sync.dma_start(
                            out=t[:, m * KH * P:(m + 1) * KH * P],
                            in_=rows[:, m * KH * P:(m + 1) * KH * P])
                else:
                    nc.sync.dma_start(out=t[:, :WS1], in_=rows[:, :WS1])
                w_sb.append(t)
            for th in range(NTH):
                rows = wall[th * P:(th + 1) * P, :]
                nc.sync.dma_start(out=w_sb[th][:, WS1:], in_=rows[:, WS1:])

            def w1sl(th, m, k):
                return w_sb[th][:, m * KH * P + k * P:m * KH * P + (k + 1) * P]

            def w2sl(th, m, ht):
                base = WS1 + m * H + ht * P
                return w_sb[th][:, base:base + P]

            for ci in range(1, len(cbs)):
                load_x(ci)

            for ci, (coff, ck) in enumerate(cbs):
                last_chunk = ci == len(cbs) - 1

                # 4 full-bank y accumulators, reused by the two GEMM2
                # half-H passes
                def ytile(q, half):
                    return psyp.tile([P, 512], FP32, tag=f"y{q}",
                                     name=f"y{q}_{ci}_{half}")

                # evictions go to DVE (ScalarE is the gelu critical path)
                # into a staging tile; one store DMA per half-H pass
                def o4tile(half):
                    return op.tile([P, 4, 512], FP32, tag="o4",
                                   name=f"o4_{ci}_{half}")

                # GEMM1 phase: h for all 8 slabs staged in SBUF
                hL = hp.tile([P, NTH, MF, 512], BF16, tag="h",
                             name=f"h_{ci}")
                for th in range(NTH):
                    for m in range(MF):
                        pt = ps1p.tile([P, 512], FP32, tag="ps1")
                        for k in range(KH):
                            nc.tensor.matmul(
                                pt[:, :ck],
                                w1sl(th, m, k),
                                x_t[ci][:, k, :],
                                start=(k == 0), stop=(k == KH - 1),
                            )
                        nc.scalar.activation(
                            hL[:, th, m, :ck], pt[:, :ck], GELU,
                            bias=b1_sb[:, th * MF + m:th * MF + m + 1],
                        )

                # GEMM2: two passes of 4 H-tiles. th outer inside a pass
                # so the w2 slabs stream at the DMA-sustainable rate on
                # chunk 0; on the final chunk's second pass, ht outer so
                # each accumulator closes early and evictions + stores
                # chase the remaining matmuls
                for half in range(2):
                    y_q = [ytile(q, half) for q in range(4)]
                    o4 = o4tile(half)

                    def evict(q):
                        nc.vector.tensor_scalar_add(
                            o4[:, q, :ck], y_q[q][:, :ck],
                            b2_sb[:, 4 * half + q:4 * half + q + 1])

                    if last_chunk and half == 1:
                        # per-ht stores: a single merged store at the very
                        # end exposes its whole ~700KB transfer (~4us)
                        for q in range(4):
                            ht = 4 * half + q
                            for th in range(NTH):
                                for m in range(MF):
                                    nc.tensor.matmul(
                                        y_q[q][:, :ck],
                                        w2sl(th, m, ht),
                                        hL[:, th, m, :ck],
                                        start=(th == 0 and m == 0),
                                        stop=(th == NTH - 1 and m == MF - 1),
                                    )
                            evict(q)
                            nc.sync.dma_start(
                                out=out_v[:, ht:ht + 1, coff:coff + ck],
                                in_=o4[:, q:q + 1, :ck])
                        continue
                    else:
                        for th in range(NTH):
                            for m in range(MF):
                                for q in range(4):
                                    nc.tensor.matmul(
                                        y_q[q][:, :ck],
                                        w2sl(th, m, 4 * half + q),
                                        hL[:, th, m, :ck],
                                        start=(th == 0 and m == 0),
                                        stop=(th == NTH - 1 and m == MF - 1),
                                    )
                        for q in range(4):
                            evict(q)
                    nc.sync.dma_start(
                        out=out_v[:, 4 * half:4 * half + 4, coff:coff + ck],
                        in_=o4[:, :, :ck])

    nc.compile()
    return nc


def _route(x: np.ndarray, router_w: np.ndarray):
    """Host router: top-2 expert ids + softmax gates per token."""
    logits = x @ router_w.T                                   # [T, E]
    top_i = np.argsort(-logits, axis=1, kind="stable")[:, :TOPK]
    top_v = np.take_along_axis(logits, top_i, axis=1)
    mx = top_v.max(axis=1, keepdims=True)
    ex = np.exp(top_v - mx)
    rw = ex / ex.sum(axis=1, keepdims=True)
    return top_i, rw.astype(np.float32)


def _swizzle_wall(w1e: np.ndarray, w2e: np.ndarray) -> np.ndarray:
    # w1 [F, H] -> rows th*128+p, cols m*1024 + k*128 + f2
    a = w1e.reshape(NTH, MF, P, KH, P).transpose(0, 4, 1, 3, 2).reshape(
        NTH * P, KH * FT)
    # w2 [H, F] -> w2.T [F, H] -> rows th*128+p, cols m*1024+h
    b = w2e.T.reshape(NTH, MF, P, H).transpose(0, 2, 1, 3).reshape(
        NTH * P, MF * H)
    return np.ascontiguousarray(
        np.concatenate([a, b], axis=1)).astype(ml_dtypes.bfloat16)


def kernel(hidden_states, router_w, w1, b1, w2, b2):
    hidden_states = np.ascontiguousarray(np.asarray(hidden_states, np.float32))
    router_w = np.ascontiguousarray(np.asarray(router_w, np.float32))
    w1 = np.asarray(w1, np.float32)
    b1 = np.asarray(b1, np.float32)
    w2 = np.asarray(w2, np.float32)
    b2 = np.asarray(b2, np.float32)

    B, S, _ = hidden_states.shape
    T = B * S
    x = hidden_states.reshape(T, H)

    top_i, rw = _route(x, router_w)

    sel_idx = []
    sel_gate = []
    for e in range(E):
        mask = top_i == e                                     # [T, K]
        rows = np.nonzero(mask.any(axis=1))[0]
        g = rw[rows[:, None], np.argmax(mask[rows], axis=1)[:, None]][:, 0]
        sel_idx.append(rows)
        sel_gate.append(g.astype(np.float32))

    # One job per (expert, token-chunk). Normally each expert fits in one
    # chunk and a single 8-core SPMD round runs everything; with an extreme
    # routing skew an expert's batch is split into <=C_MAX chunks (bounded
    # by SBUF) and extra rounds run.
    C_MAX = 2048
    jobs = []                                   # (expert, rows, gates)
    for e in range(E):
        rows, g = sel_idx[e], sel_gate[e]
        for off in range(0, max(len(rows), 1), C_MAX):
            jobs.append((e, rows[off:off + C_MAX], g[off:off + C_MAX]))

    n_rounds = -(-len(jobs) // N_CORES)
    cmax = max(len(r) for _, r, _ in jobs)
    C = max(P, -(-cmax // 8) * 8)

    if C not in _cache:
        _cache[C] = _build(C)
    nc = _cache[C]
    cbs = _chunks(C)

    w_bf = {}
    def expert_inputs(e):
        if e not in w_bf:
            w_bf[e] = {
                "wall": _swizzle_wall(w1[e], w2[e]),
                "b1c": np.ascontiguousarray(b1[e].reshape(F // P, P).T),
                "b2c": np.ascontiguousarray(b2[e].reshape(H // P, P).T),
            }
        return w_bf[e]

    global LAST_EXEC_TIME_NS
    LAST_EXEC_TIME_NS = 0
    out = np.zeros((T, H), np.float32)
    for r in range(n_rounds):
        batch = jobs[r * N_CORES:(r + 1) * N_CORES]
        while len(batch) < N_CORES:
            batch.append((0, sel_idx[0][:0], sel_gate[0][:0]))
        in_maps = []
        for e, rows, g in batch:
            n_e = len(rows)
            xpad = np.zeros((C, H), np.float32)
            xpad[:n_e] = x[rows]
            # chunk-major swizzle: col 8*coff + k*ck + c = x[coff+c, k*128+p]
            xc = np.concatenate(
                [xpad[coff:coff + ck].reshape(ck, KH, P)
                 .transpose(2, 1, 0).reshape(P, KH * ck)
                 for coff, ck in cbs], axis=1)
            in_maps.append({
                "xc": np.ascontiguousarray(xc).astype(ml_dtypes.bfloat16),
                **expert_inputs(e),
            })

        res = run_bass_kernel_spmd(nc, in_maps, list(range(N_CORES)), trace=TRACE)
        if res.exec_time_ns:
            LAST_EXEC_TIME_NS += res.exec_time_ns

        for core, (e, rows, g) in enumerate(batch):
            if len(rows):
                # y comes back [H, C] with b2 folded in; gate + top-2
                # combine on host (row indices unique within one job)
                out[rows] += g[:, None] * res.results[core]["out"][:, :len(rows)].T
    return out.reshape(B, S, H)
